# revision 10
# baseline (speedup 1.0000x reference)
"""Trainium2 Bass kernel for nn_LocalInteractionLayer (sparse_attention).

Math (after untangling the reference's raw reshape):
  q = x @ Wq.T + bq                      [B,S,HD]
  k/v computed on front-zero-padded x    [B,S+15,HD]
  scores[b,s,h,w] = <q[b,s,h*64:(h+1)*64], k[b,s+h, w*64:(w+1)*64]> / 8
  attn = softmax over w (16 chunks)
  out[b,s,h*64+df] = sum_w attn[b,s,h,w] * v[b,s+h, w*64+df]

Sharding: 8 cores = 4 batches x 2 sequence halves (1024 rows each,
15-row halo on the key/value side).

Per-core device program (identical SPMD, different inputs):
  - PE: the three projections in bf16 ([pos,hd] tiles, bias folded in via a
    ones-row matmul into the PSUM accumulation group).
  - Attention runs "key-major": partition = key row. Per head the query tile
    is reloaded shifted by h via small DMAs from a q scratch DRAM tensor.
  - DVE: score products (TT mul, 2x bf16), in-place halving-tree adds for the
    64->1 and 16->1 segment sums (tensor_reduce is 1x-only, trees are 2x),
    softmax arithmetic. ACT: PSUM evacuation casts + exp.
  - Output pieces shifted back to query-major rows with small DMAs.
"""

import os
import sys

import numpy as np

for _p in ("/opt/trn_rl_repo", "/opt/trn_rl_repo/concourse"):
    if _p not in sys.path and os.path.isdir(_p):
        sys.path.insert(0, _p)

import ml_dtypes

import concourse.bass as bass
import concourse.tile as tile
from concourse import mybir
from concourse.bass_utils import run_bass_kernel_spmd

BF16 = mybir.dt.bfloat16
F32 = mybir.dt.float32

B, S, D = 4, 2048, 1024
WIN, H, DF = 16, 16, 64
HD = H * DF
SH = S // 2            # per-core sequence rows
HALO = WIN - 1         # 15
NROWS = SH + HALO      # 1039 rows of x needed per core
NPAD = 1152            # 9 * 128, padded halo rows
NT = NPAD // 128       # 9 projection tiles / key tiles
QDRAM_ROWS = 1280      # q scratch rows (shifted reads may touch up to 1167)

_CACHE = {}


def build_nc(trace_friendly: bool = False):
    """Build the per-core Bass program (same on all 8 cores)."""
    from concourse import bacc
    nc = bacc.Bacc("TRN2", target_bir_lowering=False, debug=False, num_devices=8)

    xT = nc.dram_tensor("xT", [D, NPAD], BF16, kind="ExternalInput")
    wT = nc.dram_tensor("wT", [3, D, HD], BF16, kind="ExternalInput")
    biases = nc.dram_tensor("biases", [3, HD], BF16, kind="ExternalInput")
    out = nc.dram_tensor("out", [SH, HD], F32, kind="ExternalOutput")

    with tile.TileContext(nc) as tc:
        _build_tile(tc, xT, wT, biases, out)
    nc.finalize()
    return nc


def _build_tile(tc, xT, wT, biases, out):
    nc = tc.nc
    from contextlib import ExitStack

    with ExitStack() as ctx:
        consts = ctx.enter_context(tc.tile_pool(name="consts", bufs=1))
        xpool = ctx.enter_context(tc.tile_pool(name="xpool", bufs=3))
        ppool = ctx.enter_context(tc.tile_pool(name="ppool", bufs=2, space="PSUM"))
        evac = ctx.enter_context(tc.tile_pool(name="evac", bufs=3))
        att_big = ctx.enter_context(tc.tile_pool(name="att_big", bufs=1))
        att_sm = ctx.enter_context(tc.tile_pool(name="att_sm", bufs=2))

        # ---- static SBUF ----
        w_sb = consts.tile([128, 3, 8, HD], BF16)       # 48KB/part
        k_sb = consts.tile([128, NT, HD], BF16)         # 18KB/part
        v_sb = consts.tile([128, NT, HD], BF16)         # 18KB/part
        q_sb = consts.tile([128, NT, HD], BF16)         # 18KB/part
        bias_sb = consts.tile([1, 3, HD], BF16)
        ones_sb = consts.tile([1, 128], BF16)

        for t in range(3):
            for dc in range(8):
                nc.sync.dma_start(
                    out=w_sb[:, t, dc, :], in_=wT[t, dc * 128:(dc + 1) * 128, :]
                )
        nc.sync.dma_start(out=bias_sb[0:1, :, :], in_=biases[:, :])
        nc.vector.memset(ones_sb[:], 1.0)

        # ---- projections ----
        for t in range(NT):
            xt = xpool.tile([128, 8, 128], BF16)
            for dc in range(8):
                nc.sync.dma_start(
                    out=xt[:, dc, :], in_=xT[dc * 128:(dc + 1) * 128, t * 128:(t + 1) * 128]
                )
            for p in range(3):  # 0=q, 1=k, 2=v
                ps = ppool.tile([128, HD], F32, tag="ps")
                for n0 in (0, 512):
                    nc.tensor.matmul(
                        ps[:, n0:n0 + 512],
                        lhsT=ones_sb[:, :],
                        rhs=bias_sb[:, p, n0:n0 + 512],
                        start=True, stop=False,
                    )
                    for dc in range(8):
                        nc.tensor.matmul(
                            ps[:, n0:n0 + 512],
                            lhsT=xt[:, dc, :],
                            rhs=w_sb[:, p, dc, n0:n0 + 512],
                            start=False, stop=(dc == 7),
                        )
                if p == 0:
                    nc.scalar.copy(q_sb[:, t, :], ps[:])
                elif p == 1:
                    nc.scalar.copy(k_sb[:, t, :], ps[:])
                else:
                    nc.scalar.copy(v_sb[:, t, :], ps[:])

        # ---- attention, key-major over 9 key tiles ----
        for j in range(NT):
            qsh = att_sm.tile([128, H, DF], BF16, tag="qsh")
            if j == NT - 1:
                # tail tile: some (pi, h) slots have no q row (sigma >= SH,
                # never stored) — define them so the race/uninit checks pass
                nc.vector.memset(qsh[:], 0.0)
            for h in range(H):
                off = HALO - h  # shifted window starts `off` rows into block j
                nc.sync.dma_start(
                    out=qsh[: 128 - off, h, :],
                    in_=q_sb[off:128, j, h * DF:(h + 1) * DF],
                )
                if off > 0 and j + 1 < NT:
                    nc.sync.dma_start(
                        out=qsh[128 - off:128, h, :],
                        in_=q_sb[0:off, j + 1, h * DF:(h + 1) * DF],
                    )
                # j == NT-1 tail: slots that would read block NT keep stale
                # data; those slots map to sigma >= SH and are never stored.

            # stage A: prodA[p, h, w, f] = k[p, w*64+f] * qsh[p, h, f]
            prodA = att_big.tile([128, H, WIN, DF], BF16, tag="prodA")
            kb = k_sb[:, j, :]
            k_view = bass.AP(
                tensor=kb.tensor, offset=kb.offset,
                ap=[list(kb.ap[0]), [0, H], [DF, WIN], [1, DF]],
            )
            q_view = bass.AP(
                tensor=qsh.tensor, offset=qsh.offset,
                ap=[list(qsh.ap[0]), [DF, H], [0, WIN], [1, DF]],
            )
            nc.vector.tensor_mul(prodA[:], k_view, q_view)

            # in-place halving tree over df: 64 -> 1
            sz = DF // 2
            while sz >= 1:
                if sz > 1:
                    nc.vector.tensor_add(
                        prodA[:, :, :, 0:sz],
                        prodA[:, :, :, 0:sz],
                        prodA[:, :, :, sz:2 * sz],
                    )
                else:
                    scr = att_sm.tile([128, H, WIN], F32, tag="scr")
                    nc.vector.tensor_add(
                        scr[:], prodA[:, :, :, 0], prodA[:, :, :, 1]
                    )
                sz //= 2

            # softmax over w (16)
            nmax = att_sm.tile([128, H], F32, tag="nmax")
            nc.vector.reduce_max(nmax[:], scr[:], axis=mybir.AxisListType.X, negate=True)
            e = att_sm.tile([128, H, WIN], F32, tag="e")
            nmax_b = bass.AP(
                tensor=nmax.tensor, offset=nmax.offset,
                ap=[list(nmax.ap[0]), [1, H], [0, WIN]],
            )
            nc.vector.tensor_add(e[:], scr[:], nmax_b)
            nc.scalar.activation(e[:], e[:], mybir.ActivationFunctionType.Exp)
            ssum = att_sm.tile([128, H], F32, tag="ssum")
            nc.vector.reduce_sum(ssum[:], e[:], axis=mybir.AxisListType.X)
            nc.vector.reciprocal(ssum[:], ssum[:])
            attn = att_sm.tile([128, H, WIN], BF16, tag="attn")
            ssum_b = bass.AP(
                tensor=ssum.tensor, offset=ssum.offset,
                ap=[list(ssum.ap[0]), [1, H], [0, WIN]],
            )
            nc.vector.tensor_mul(attn[:], e[:], ssum_b)

            # stage C: prodC[p, h, f, w] = v[p, f*16+w] * attn[p, h, w]
            prodC = att_big.tile([128, H, DF, WIN], BF16, tag="prodC")
            vb = v_sb[:, j, :]
            v_view = bass.AP(
                tensor=vb.tensor, offset=vb.offset,
                ap=[list(vb.ap[0]), [0, H], [WIN, DF], [1, WIN]],
            )
            a_view = bass.AP(
                tensor=attn.tensor, offset=attn.offset,
                ap=[list(attn.ap[0]), [WIN, H], [0, DF], [1, WIN]],
            )
            nc.vector.tensor_mul(prodC[:], v_view, a_view)

            sz = WIN // 2
            while sz >= 1:
                if sz > 1:
                    nc.vector.tensor_add(
                        prodC[:, :, :, 0:sz],
                        prodC[:, :, :, 0:sz],
                        prodC[:, :, :, sz:2 * sz],
                    )
                else:
                    tmp = att_sm.tile([128, H, DF], F32, tag="tmp")
                    nc.vector.tensor_add(
                        tmp[:], prodC[:, :, :, 0], prodC[:, :, :, 1]
                    )
                sz //= 2

            # shift back to query-major rows: sigma = 128*j + pi - h
            for h in range(H):
                if j == 0:
                    if h == 0:
                        src = tmp[:, h, :]
                        dst = out[0:128, h * DF:(h + 1) * DF]
                    else:
                        src = tmp[h:128, h, :]
                        dst = out[0:128 - h, h * DF:(h + 1) * DF]
                elif j < NT - 1:
                    src = tmp[:, h, :]
                    dst = out[j * 128 - h:j * 128 + 128 - h, h * DF:(h + 1) * DF]
                else:
                    if h == 0:
                        continue
                    src = tmp[0:h, h, :]
                    dst = out[SH - h:SH, h * DF:(h + 1) * DF]
                nc.sync.dma_start(out=dst, in_=src)


def _host_prep(input_seq, Wq, bq, Wk, bk, Wv, bv):
    """Build the 8 per-core input maps."""
    input_seq = np.asarray(input_seq, dtype=np.float32)
    Wq = np.asarray(Wq, dtype=np.float32)
    Wk = np.asarray(Wk, dtype=np.float32)
    Wv = np.asarray(Wv, dtype=np.float32)
    bq = np.asarray(bq, dtype=np.float32)
    bk = np.asarray(bk, dtype=np.float32)
    bv = np.asarray(bv, dtype=np.float32)

    scale = 1.0 / np.sqrt(DF)
    # v column permutation: new col (df*16 + w) = old col (w*64 + df)
    perm = (np.arange(HD).reshape(WIN, DF).T).reshape(-1)

    wT = np.stack([
        (Wq.T * scale),
        Wk.T,
        (Wv.T)[:, perm],
    ]).astype(ml_dtypes.bfloat16)                    # [3, D, HD]
    biases = np.stack([
        bq * scale,
        bk,
        bv[perm],
    ]).astype(ml_dtypes.bfloat16)                    # [3, HD]

    in_maps = []
    for c in range(8):
        b, half = c // 2, c % 2
        s0 = half * SH
        xh = np.zeros((NPAD, D), dtype=np.float32)
        lo = s0 - HALO
        src_lo = max(lo, 0)
        xh[src_lo - lo: src_lo - lo + (s0 + SH - src_lo)] = input_seq[b, src_lo: s0 + SH]
        xT = np.ascontiguousarray(xh.T).astype(ml_dtypes.bfloat16)
        in_maps.append({"xT": xT, "wT": wT, "biases": biases})
    return in_maps


def _get_nc():
    if "nc" not in _CACHE:
        _CACHE["nc"] = build_nc()
    return _CACHE["nc"]


def _ensure_ntff_hook():
    """Register the axon NTFF profile hook if the image's antenv lacks it."""
    import types
    try:
        from antenv.axon_hooks import get_axon_ntff_profile_hook  # noqa: F401
        return
    except ImportError:
        pass
    try:
        import antenv
        mod = types.ModuleType("antenv.axon_hooks")
        _state = {"hook": None}
        mod.set_axon_ntff_profile_hook = lambda h: _state.__setitem__("hook", h)
        mod.get_axon_ntff_profile_hook = lambda: _state["hook"]
        sys.modules["antenv.axon_hooks"] = mod
        antenv.axon_hooks = mod
        boot_dir = "/root/.axon_site/trn_agent_boot"
        if boot_dir not in sys.path and os.path.isdir(boot_dir):
            sys.path.insert(0, boot_dir)
        import trn_boot
        hook = trn_boot._ntff_profile_via_ctypes("/opt/axon/libaxon_pjrt.so")
        if hook is not None:
            mod.set_axon_ntff_profile_hook(hook)
    except Exception as e:  # profiling is best-effort
        print(f"ntff hook setup failed: {e}")


def kernel(input_seq, Wq, bq, Wk, bk, Wv, bv, trace=False, **trace_kwargs):
    if trace:
        _ensure_ntff_hook()
    nc = _get_nc()
    in_maps = _host_prep(input_seq, Wq, bq, Wk, bk, Wv, bv)
    res = run_bass_kernel_spmd(nc, in_maps, list(range(8)), trace=trace, **trace_kwargs)
    out = np.empty((B, S, HD), dtype=np.float32)
    for c in range(8):
        b, half = c // 2, c % 2
        out[b, half * SH:(half + 1) * SH] = res.results[c]["out"]
    if trace:
        return out, res
    return out
